# revision 13
# baseline (speedup 1.0000x reference)
"""ChebyKAN layer kernel for 8 Trainium2 NeuronCores.

Reference computation:
    t = tanh(clip(x, -10, 10))                       # [N, I]
    ch = stack([T0(t) .. T4(t)], -1)                  # Chebyshev basis, deg 4
    out = clip(einsum('nid,oid->no', ch, coeffs), -10, 10)

Since t = tanh(.) lies in (-1, 1), every Chebyshev value is in [-1, 1] and the
intermediate clips at +-10 are no-ops.  We rewrite the basis as
{t, v, t*v, v^2} with v = 2t^2-1 on the host:

    out[n,o] = bias[o] + sum_i ( (c1-c3) t + c2 v + 2 c3 (tv) + 2 c4 v^2 )
    bias[o] = sum_i (c0 - c4)[o,i]          (added on host)

which is a [N, 4*I] x [4*I, O] matmul after the elementwise basis prep.

Mixed precision: the tolerance (norm rel err < 2e-2) lets a fraction of the
contraction run as e4m3 DoubleRow matmuls (2 fp8 MACs/cell/cycle).  Measured
quantization noise per plane ranks {v, tv, t, v^2} cheapest-first; 12 of 32
pair-units (plane v entirely + the first half of plane tv) in fp8 lands at
~1.76e-2 total.  The rest stays fp16.  All weights are prescaled by 2^13 so
the fp8 weights sit in e4m3's normal range; the PSUM result is scaled back
by 2^-13 during evacuation (free - the evacuation copy becomes a scaled copy).

Sharding over 8 cores: 4-way over N (rows of x), 2-way over O (output
columns).  Each core holds its W shard resident in SBUF and streams 32
tiles of 128 rows of x, computing tanh + powers on scalar/vector engines
and the matmul on the tensor engine (fp32 PSUM accumulation).
"""

import numpy as np

N, I, O, DEG = 16384, 2048, 2048, 4
NB, OB = 4, 2                      # core grid: 4-way over N, 2-way over O
NSH = N // NB                      # 4096 rows per core
OSH = O // OB                      # 1024 out cols per core
NT = NSH // 128                    # 32 n-tiles per core
IB = I // 128                      # 16 i-blocks

# bf16/fp16 contraction tiles: (plane, i-block) in matmul order
K16 = [(0, ib) for ib in range(IB)] + [(2, ib) for ib in range(14, IB)] + \
      [(3, ib) for ib in range(IB)]                     # 34 tiles
# fp8 DoubleRow pair-units: (plane, pair j) covering i-blocks 2j, 2j+1
K8 = [(1, j) for j in range(8)] + [(2, j) for j in range(7)]  # 15 units
NK16 = len(K16)
NK8 = len(K8)
WS = 2.0 ** 13                     # weight prescale for the fp8 path


def _build_program():
    from concourse.bacc import Bacc
    from concourse.tile import TileContext
    import concourse.mybir as mybir

    f32 = mybir.dt.float32
    f16 = mybir.dt.float16
    f8 = mybir.dt.float8e4
    TANH = mybir.ActivationFunctionType.Tanh
    DR = mybir.MatmulPerfMode.DoubleRow

    nc = Bacc(None, target_bir_lowering=False)
    xt_d = nc.dram_tensor("xt", [NT, 128, I], f32, kind="ExternalInput")
    wt_d = nc.dram_tensor("wt", [NK16, 128, OSH], f16, kind="ExternalInput")
    w8_d = nc.dram_tensor("w8", [NK8, 128, 2 * OSH], f8, kind="ExternalInput")
    out_d = nc.dram_tensor("out", [NT, 128, OSH], f32, kind="ExternalOutput")

    NWARM = 2  # n-tiles processed k-major-interleaved while W streams in
    EVAC = 2.0 ** -13

    with TileContext(nc) as tc:
        with (
            tc.tile_pool(name="wpool", bufs=1) as wpool,
            tc.tile_pool(name="xin", bufs=3) as xpool,
            tc.tile_pool(name="work", bufs=2) as pool,
            tc.tile_pool(name="tpool", bufs=3) as tpool,
            tc.tile_pool(name="psum", bufs=8, space="PSUM") as pp,
        ):
            def load_powers(nt):
                # Basis planes {t, v, t*v, v^2} with v = 2t^2-1: all bounded
                # by 1.  fp16 for the K16 tiles, e4m3 for the K8 units.
                xt = xpool.tile([128, I], f32, tag="xt")
                nc.sync.dma_start(out=xt[:], in_=xt_d[nt])
                # t = tanh(x) straight to fp16 (ACT computes fp32
                # internally) - shortest path to the first matmul.
                t1 = tpool.tile([128, I], f16, tag="t1")
                nc.scalar.activation(t1[:], xt[:], TANH)
                # t = tanh(x), in place (fp32) for the v/tv planes
                nc.scalar.activation(xt[:], xt[:], TANH)
                uv = xpool.tile([128, I], f32, tag="uv")
                nc.vector.tensor_mul(uv[:], xt[:], xt[:])
                # v = 2u - 1, in place (fp32)
                nc.vector.tensor_scalar(
                    uv[:], uv[:], 2.0, -1.0,
                    mybir.AluOpType.mult, mybir.AluOpType.add,
                )
                # plane v entirely in fp8 (pair-sliced 3D for DoubleRow)
                a8v = tpool.tile([128, IB, 128], f8, tag="a8v")
                nc.vector.tensor_copy(a8v[:, :, :], uv[:])
                # plane tv: first 14 i-blocks fp8, rest fp16
                a8tv = tpool.tile([128, 14, 128], f8, tag="a8tv")
                nc.vector.tensor_mul(a8tv[:, :, :], xt[:, 0:1792], uv[:, 0:1792])
                t3 = tpool.tile([128, 256], f16, tag="t3")
                nc.vector.tensor_mul(t3[:], xt[:, 1792:2048], uv[:, 1792:2048])
                # plane v^2 fp16
                t4 = tpool.tile([128, I], f16, tag="t4")
                nc.vector.tensor_mul(t4[:], uv[:], uv[:])
                return {"t1": t1, "t3": t3, "t4": t4, "a8v": a8v, "a8tv": a8tv}

            def lhs16(tp, k):
                p, ib = K16[k]
                if p == 0:
                    return tp["t1"][:, ib * 128:(ib + 1) * 128]
                if p == 2:
                    return tp["t3"][:, (ib - 14) * 128:(ib - 13) * 128]
                return tp["t4"][:, ib * 128:(ib + 1) * 128]

            def lhs8(tp, u):
                p, j = K8[u]
                src = tp["a8v"] if p == 1 else tp["a8tv"]
                return src[:, 2 * j:2 * j + 2, :]

            def mm16(ps, tp, k, oc, start, stop):
                nc.tensor.matmul(
                    ps[:], lhs16(tp, k),
                    wtiles[k][:, oc * 512:(oc + 1) * 512],
                    start=start, stop=stop,
                )

            def mm8(ps, tp, u, oc, start, stop):
                nc.tensor.matmul(
                    ps[:], lhs8(tp, u),
                    w8tiles[u][:, :, oc * 512:(oc + 1) * 512],
                    start=start, stop=stop, perf_mode=DR,
                )

            def store_out_oc(nt, osb, oc):
                nc.sync.dma_start(
                    out=out_d[nt, :, oc * 512:(oc + 1) * 512],
                    in_=osb[:, oc * 512:(oc + 1) * 512],
                )

            # HAM pre-warm sized to ABUT the real stream: the burst must
            # still be running when t1 lands (~15.5us) or the free-running
            # MID window re-throttles the clock gate during the idle gap.
            junk = pool.tile([128, 512], f16, tag="junk")
            nc.vector.memset(junk[:], 0.0)
            ps_j = pp.tile([128, 512], f32, tag="ps")
            NJUNK = 27  # burst ends right at t1-readiness (~15.6us)
            for i in range(NJUNK):
                nc.tensor.matmul(
                    ps_j[:], junk[:, 0:128], junk[:],
                    start=(i == 0), stop=(i == NJUNK - 1),
                )

            # x tiles for the warmup n-tiles first so their DMAs aren't
            # queued behind the ~13 MB of W.
            tps = []
            for wnt in range(NWARM):
                tp_w = load_powers(wnt)
                tps.append(tp_w)

            wtiles = []
            for k in range(NK16):
                w = wpool.tile([128, OSH], f16, tag=f"w{k}")
                nc.sync.dma_start(out=w[:], in_=wt_d[k])
                wtiles.append(w)
            w8tiles = []
            for u in range(NK8):
                w = wpool.tile([128, 2, OSH], f8, tag=f"w8_{u}")
                nc.sync.dma_start(out=w[:, :, :], in_=w8_d[u])
                w8tiles.append(w)

            # Warmup phase: k-major across NWARM*2 psum groups, so the PE has
            # work for every W k-tile as it lands instead of idling until the
            # whole W shard is resident.
            groups = [(nt, oc) for nt in range(NWARM) for oc in range(OSH // 512)]
            pss = {}
            for g in groups:
                ps_tile = pp.tile([128, 512], f32, tag="ps")
                pss[g] = ps_tile
            for k in range(NK16):
                for (nt, oc) in groups:
                    mm16(pss[(nt, oc)], tps[nt], k, oc,
                         start=(k == 0), stop=False)
            for u in range(NK8):
                for (nt, oc) in groups:
                    mm8(pss[(nt, oc)], tps[nt], u, oc,
                        start=False, stop=(u == NK8 - 1))
            # Prep the first steady n-tile BEFORE the warmup evacuations hit
            # the in-order scalar queue: its tanh then runs during the late
            # warmup matmuls instead of queueing behind evacs that wait for
            # them, closing the warmup->steady PE bubble.
            tps.append(load_powers(NWARM))
            for nt in range(NWARM):
                osb = pool.tile([128, OSH], f32, tag="osb")
                for oc in range(OSH // 512):
                    nc.scalar.mul(osb[:, oc * 512:(oc + 1) * 512],
                                  pss[(nt, oc)][:], EVAC)
                    store_out_oc(nt, osb, oc)

            # Steady state: W fully resident, k-outer so each stationary
            # t-slice feeds both oc matmuls back to back.
            NOC = OSH // 512
            for nt in range(NWARM, NT):
                tp = tps[nt] if nt < len(tps) else load_powers(nt)
                osb = pool.tile([128, OSH], f32, tag="osb")
                pss2 = []
                for _ in range(NOC):
                    ps = pp.tile([128, 512], f32, tag="ps")
                    pss2.append(ps)
                if nt < NT - 1:
                    for k in range(NK16):
                        for oc in range(NOC):
                            mm16(pss2[oc], tp, k, oc, start=(k == 0), stop=False)
                    for u in range(NK8):
                        for oc in range(NOC):
                            mm8(pss2[oc], tp, u, oc, start=False,
                                stop=(u == NK8 - 1))
                    for oc in range(NOC):
                        nc.scalar.mul(osb[:, oc * 512:(oc + 1) * 512],
                                      pss2[oc][:], EVAC)
                        store_out_oc(nt, osb, oc)
                else:
                    # Last tile goes group-sequential: oc0's evacuation and
                    # store hide under oc1's matmuls, shortening the tail.
                    for oc in range(NOC):
                        for k in range(NK16):
                            mm16(pss2[oc], tp, k, oc, start=(k == 0), stop=False)
                        for u in range(NK8):
                            mm8(pss2[oc], tp, u, oc, start=False,
                                stop=(u == NK8 - 1))
                        nc.scalar.mul(osb[:, oc * 512:(oc + 1) * 512],
                                      pss2[oc][:], EVAC)
                        store_out_oc(nt, osb, oc)

    nc.finalize()
    return nc


def _prep_inputs(x, coeffs):
    """Host-side shard prep: transposed/tiled x per N-block, packed fp16 +
    e4m3 weights per O-block, and the T0/bias term."""
    import ml_dtypes

    # Basis on device: {t, v, t*v, v^2} with v = 2t^2 - 1 (= T2).  Then
    # T1 = t, T2 = v, T3 = 2(tv) - t, T4 = 2v^2 - 1, so
    # out = (c0 - c4) + (c1 - c3) t + c2 v + 2 c3 (tv) + 2 c4 v^2.
    c = coeffs.astype(np.float64)
    w_mono = np.stack(
        [
            c[..., 1] - c[..., 3],    # t
            c[..., 2],                # v
            2.0 * c[..., 3],          # t*v
            2.0 * c[..., 4],          # v^2
        ]
    ) * WS  # [4, O, I], prescaled
    bias = (c[..., 0] - c[..., 4]).sum(axis=1)  # [O] float64

    xparts = []
    for nb in range(NB):
        xs = x[nb * NSH:(nb + 1) * NSH, :]                 # [NSH, I]
        # [nt, n_in, i_blk, i_in] -> [nt, i_in, i_blk, n_in]
        xp = xs.reshape(NT, 128, IB, 128).transpose(0, 3, 2, 1)
        xparts.append(np.ascontiguousarray(xp.reshape(NT, 128, I), dtype=np.float32))

    wparts, w8parts = [], []
    for ob in range(OB):
        wsl = w_mono[:, ob * OSH:(ob + 1) * OSH, :]        # [4, OSH, I]
        # fp16 tiles: [k, i_in, o]
        wt = np.empty((NK16, 128, OSH), dtype=np.float16)
        for k, (p, ib) in enumerate(K16):
            wt[k] = wsl[p, :, ib * 128:(ib + 1) * 128].T
        wparts.append(wt)
        # fp8 DoubleRow units: [u, i_in, slot*OSH + o]
        w8 = np.empty((NK8, 128, 2 * OSH), dtype=ml_dtypes.float8_e4m3)
        for u, (p, j) in enumerate(K8):
            for s in range(2):
                blk = wsl[p, :, (2 * j + s) * 128:(2 * j + s + 1) * 128].T
                w8[u, :, s * OSH:(s + 1) * OSH] = blk.astype(np.float32)
        w8parts.append(w8)

    return xparts, wparts, w8parts, bias


def _run(x, coeffs, trace=False):
    import os

    from concourse.bass_utils import run_bass_kernel_spmd

    if not trace:
        # A stray BASS_TRACE in the environment would route through the NTFF
        # profile hook, which this image does not ship.
        os.environ["BASS_NEVER_TRACE"] = "1"
    else:
        os.environ.pop("BASS_NEVER_TRACE", None)

    xparts, wparts, w8parts, bias = _prep_inputs(x, coeffs)
    nc = _build_program()
    in_maps = [
        {"xt": xparts[c // OB], "wt": wparts[c % OB], "w8": w8parts[c % OB]}
        for c in range(NB * OB)
    ]
    res = run_bass_kernel_spmd(nc, in_maps, list(range(NB * OB)), trace=trace)

    out = np.empty((N, O), dtype=np.float64)
    for c in range(NB * OB):
        nb, ob = divmod(c, OB)
        out[nb * NSH:(nb + 1) * NSH, ob * OSH:(ob + 1) * OSH] = (
            res.results[c]["out"].reshape(NSH, OSH)
        )
    out += bias[None, :]
    np.clip(out, -10.0, 10.0, out=out)
    return out.astype(np.float32), res


def kernel(x, coeffs):
    return _run(np.asarray(x), np.asarray(coeffs))[0]


# revision 14
# speedup vs baseline: 1.0088x; 1.0088x over previous
"""ChebyKAN layer kernel for 8 Trainium2 NeuronCores.

Reference computation:
    t = tanh(clip(x, -10, 10))                       # [N, I]
    ch = stack([T0(t) .. T4(t)], -1)                  # Chebyshev basis, deg 4
    out = clip(einsum('nid,oid->no', ch, coeffs), -10, 10)

Since t = tanh(.) lies in (-1, 1), every Chebyshev value is in [-1, 1] and the
intermediate clips at +-10 are no-ops.  We rewrite the basis as
{t, v, t*v, v^2} with v = 2t^2-1 on the host:

    out[n,o] = bias[o] + sum_i ( (c1-c3) t + c2 v + 2 c3 (tv) + 2 c4 v^2 )
    bias[o] = sum_i (c0 - c4)[o,i]          (added on host)

which is a [N, 4*I] x [4*I, O] matmul after the elementwise basis prep.

Mixed precision: the tolerance (norm rel err < 2e-2) lets a fraction of the
contraction run as e4m3 DoubleRow matmuls (2 fp8 MACs/cell/cycle).  Measured
quantization noise per plane ranks {v, tv, t, v^2} cheapest-first; 12 of 32
pair-units (plane v entirely + the first half of plane tv) in fp8 lands at
~1.76e-2 total.  The rest stays fp16.  All weights are prescaled by 2^13 so
the fp8 weights sit in e4m3's normal range; the PSUM result is scaled back
by 2^-13 during evacuation (free - the evacuation copy becomes a scaled copy).

Sharding over 8 cores: 4-way over N (rows of x), 2-way over O (output
columns).  Each core holds its W shard resident in SBUF and streams 32
tiles of 128 rows of x, computing tanh + powers on scalar/vector engines
and the matmul on the tensor engine (fp32 PSUM accumulation).
"""

import numpy as np

N, I, O, DEG = 16384, 2048, 2048, 4
NB, OB = 4, 2                      # core grid: 4-way over N, 2-way over O
NSH = N // NB                      # 4096 rows per core
OSH = O // OB                      # 1024 out cols per core
NT = NSH // 128                    # 32 n-tiles per core
IB = I // 128                      # 16 i-blocks

# bf16/fp16 contraction tiles: (plane, i-block) in matmul order
K16 = [(0, ib) for ib in range(IB)] + [(2, ib) for ib in range(14, IB)] + \
      [(3, ib) for ib in range(IB)]                     # 34 tiles
# fp8 DoubleRow pair-units: (plane, pair j) covering i-blocks 2j, 2j+1
K8 = [(1, j) for j in range(8)] + [(2, j) for j in range(7)]  # 15 units
NK16 = len(K16)
NK8 = len(K8)
WS = 2.0 ** 13                     # weight prescale for the fp8 path


def _build_program():
    from concourse.bacc import Bacc
    from concourse.tile import TileContext
    import concourse.mybir as mybir

    f32 = mybir.dt.float32
    f16 = mybir.dt.float16
    f8 = mybir.dt.float8e4
    TANH = mybir.ActivationFunctionType.Tanh
    DR = mybir.MatmulPerfMode.DoubleRow

    nc = Bacc(None, target_bir_lowering=False)
    xt_d = nc.dram_tensor("xt", [NT, 128, I], f32, kind="ExternalInput")
    wt_d = nc.dram_tensor("wt", [NK16, 128, OSH], f16, kind="ExternalInput")
    w8_d = nc.dram_tensor("w8", [NK8, 128, 2 * OSH], f8, kind="ExternalInput")
    out_d = nc.dram_tensor("out", [NT, 128, OSH], f32, kind="ExternalOutput")

    NWARM = 2  # n-tiles processed k-major-interleaved while W streams in
    EVAC = 2.0 ** -13

    with TileContext(nc) as tc:
        with (
            tc.tile_pool(name="wpool", bufs=1) as wpool,
            tc.tile_pool(name="xin", bufs=3) as xpool,
            tc.tile_pool(name="work", bufs=2) as pool,
            tc.tile_pool(name="tpool", bufs=3) as tpool,
            tc.tile_pool(name="psum", bufs=8, space="PSUM") as pp,
        ):
            def load_powers(nt):
                # Basis planes {t, v, t*v, v^2} with v = 2t^2-1: all bounded
                # by 1.  fp16 for the K16 tiles, e4m3 for the K8 units.
                xt = xpool.tile([128, I], f32, tag="xt")
                nc.sync.dma_start(out=xt[:], in_=xt_d[nt])
                # t = tanh(x) straight to fp16 (ACT computes fp32
                # internally) - shortest path to the first matmul.
                t1 = tpool.tile([128, I], f16, tag="t1")
                nc.scalar.activation(t1[:], xt[:], TANH)
                # t = tanh(x), in place (fp32) for the v/tv planes
                nc.scalar.activation(xt[:], xt[:], TANH)
                uv = xpool.tile([128, I], f32, tag="uv")
                nc.vector.tensor_mul(uv[:], xt[:], xt[:])
                # v = 2u - 1, in place (fp32)
                nc.vector.tensor_scalar(
                    uv[:], uv[:], 2.0, -1.0,
                    mybir.AluOpType.mult, mybir.AluOpType.add,
                )
                # plane v entirely in fp8 (pair-sliced 3D for DoubleRow)
                a8v = tpool.tile([128, IB, 128], f8, tag="a8v")
                nc.vector.tensor_copy(a8v[:, :, :], uv[:])
                # plane tv: first 14 i-blocks fp8, rest fp16
                a8tv = tpool.tile([128, 14, 128], f8, tag="a8tv")
                nc.vector.tensor_mul(a8tv[:, :, :], xt[:, 0:1792], uv[:, 0:1792])
                t3 = tpool.tile([128, 256], f16, tag="t3")
                nc.vector.tensor_mul(t3[:], xt[:, 1792:2048], uv[:, 1792:2048])
                # plane v^2 fp16
                t4 = tpool.tile([128, I], f16, tag="t4")
                nc.vector.tensor_mul(t4[:], uv[:], uv[:])
                return {"t1": t1, "t3": t3, "t4": t4, "a8v": a8v, "a8tv": a8tv}

            def lhs16(tp, k):
                p, ib = K16[k]
                if p == 0:
                    return tp["t1"][:, ib * 128:(ib + 1) * 128]
                if p == 2:
                    return tp["t3"][:, (ib - 14) * 128:(ib - 13) * 128]
                return tp["t4"][:, ib * 128:(ib + 1) * 128]

            def lhs8(tp, u):
                p, j = K8[u]
                src = tp["a8v"] if p == 1 else tp["a8tv"]
                return src[:, 2 * j:2 * j + 2, :]

            def mm16(ps, tp, k, oc, start, stop):
                nc.tensor.matmul(
                    ps[:], lhs16(tp, k),
                    wtiles[k][:, oc * 512:(oc + 1) * 512],
                    start=start, stop=stop,
                )

            def mm8(ps, tp, u, oc, start, stop):
                nc.tensor.matmul(
                    ps[:], lhs8(tp, u),
                    w8tiles[u][:, :, oc * 512:(oc + 1) * 512],
                    start=start, stop=stop, perf_mode=DR,
                )

            def store_out_oc(nt, osb, oc):
                nc.sync.dma_start(
                    out=out_d[nt, :, oc * 512:(oc + 1) * 512],
                    in_=osb[:, oc * 512:(oc + 1) * 512],
                )

            # HAM pre-warm sized to ABUT the real stream: the burst must
            # still be running when t1 lands (~15.5us) or the free-running
            # MID window re-throttles the clock gate during the idle gap.
            junk = pool.tile([128, 512], f16, tag="junk")
            nc.vector.memset(junk[:], 0.0)
            ps_j = pp.tile([128, 512], f32, tag="ps")
            NJUNK = 27  # burst ends right at t1-readiness (~15.6us)
            for i in range(NJUNK):
                nc.tensor.matmul(
                    ps_j[:], junk[:, 0:128], junk[:],
                    start=(i == 0), stop=(i == NJUNK - 1),
                )

            # x tiles for the warmup n-tiles first so their DMAs aren't
            # queued behind the ~13 MB of W.
            tps = []
            for wnt in range(NWARM):
                tp_w = load_powers(wnt)
                tps.append(tp_w)

            wtiles = []
            for k in range(NK16):
                w = wpool.tile([128, OSH], f16, tag=f"w{k}")
                nc.sync.dma_start(out=w[:], in_=wt_d[k])
                wtiles.append(w)
            w8tiles = []
            for u in range(NK8):
                w = wpool.tile([128, 2, OSH], f8, tag=f"w8_{u}")
                nc.sync.dma_start(out=w[:, :, :], in_=w8_d[u])
                w8tiles.append(w)

            # Warmup phase: k-major across NWARM*2 psum groups, so the PE has
            # work for every W k-tile as it lands instead of idling until the
            # whole W shard is resident.
            groups = [(nt, oc) for nt in range(NWARM) for oc in range(OSH // 512)]
            pss = {}
            for g in groups:
                ps_tile = pp.tile([128, 512], f32, tag="ps")
                pss[g] = ps_tile
            for k in range(NK16):
                for (nt, oc) in groups:
                    mm16(pss[(nt, oc)], tps[nt], k, oc,
                         start=(k == 0), stop=False)
            for u in range(NK8):
                for (nt, oc) in groups:
                    mm8(pss[(nt, oc)], tps[nt], u, oc,
                        start=False, stop=(u == NK8 - 1))
            # Prep the first steady n-tile BEFORE the warmup evacuations hit
            # the in-order scalar queue: its tanh then runs during the late
            # warmup matmuls instead of queueing behind evacs that wait for
            # them, closing the warmup->steady PE bubble.  The scheduler
            # orders queues by priority, not emission order, so shift this
            # block to just before the warmup matmuls (196 instructions).
            with tc.high_priority(offset=200):
                tps.append(load_powers(NWARM))
            for nt in range(NWARM):
                osb = pool.tile([128, OSH], f32, tag="osb")
                for oc in range(OSH // 512):
                    nc.scalar.mul(osb[:, oc * 512:(oc + 1) * 512],
                                  pss[(nt, oc)][:], EVAC)
                    store_out_oc(nt, osb, oc)

            # Steady state: W fully resident, k-outer so each stationary
            # t-slice feeds both oc matmuls back to back.
            NOC = OSH // 512
            for nt in range(NWARM, NT):
                tp = tps[nt] if nt < len(tps) else load_powers(nt)
                osb = pool.tile([128, OSH], f32, tag="osb")
                pss2 = []
                for _ in range(NOC):
                    ps = pp.tile([128, 512], f32, tag="ps")
                    pss2.append(ps)
                if nt < NT - 1:
                    for k in range(NK16):
                        for oc in range(NOC):
                            mm16(pss2[oc], tp, k, oc, start=(k == 0), stop=False)
                    for u in range(NK8):
                        for oc in range(NOC):
                            mm8(pss2[oc], tp, u, oc, start=False,
                                stop=(u == NK8 - 1))
                    for oc in range(NOC):
                        nc.scalar.mul(osb[:, oc * 512:(oc + 1) * 512],
                                      pss2[oc][:], EVAC)
                        store_out_oc(nt, osb, oc)
                else:
                    # Last tile goes group-sequential: oc0's evacuation and
                    # store hide under oc1's matmuls, shortening the tail.
                    for oc in range(NOC):
                        for k in range(NK16):
                            mm16(pss2[oc], tp, k, oc, start=(k == 0), stop=False)
                        for u in range(NK8):
                            mm8(pss2[oc], tp, u, oc, start=False,
                                stop=(u == NK8 - 1))
                        nc.scalar.mul(osb[:, oc * 512:(oc + 1) * 512],
                                      pss2[oc][:], EVAC)
                        store_out_oc(nt, osb, oc)

    nc.finalize()
    return nc


def _prep_inputs(x, coeffs):
    """Host-side shard prep: transposed/tiled x per N-block, packed fp16 +
    e4m3 weights per O-block, and the T0/bias term."""
    import ml_dtypes

    # Basis on device: {t, v, t*v, v^2} with v = 2t^2 - 1 (= T2).  Then
    # T1 = t, T2 = v, T3 = 2(tv) - t, T4 = 2v^2 - 1, so
    # out = (c0 - c4) + (c1 - c3) t + c2 v + 2 c3 (tv) + 2 c4 v^2.
    c = coeffs.astype(np.float64)
    w_mono = np.stack(
        [
            c[..., 1] - c[..., 3],    # t
            c[..., 2],                # v
            2.0 * c[..., 3],          # t*v
            2.0 * c[..., 4],          # v^2
        ]
    ) * WS  # [4, O, I], prescaled
    bias = (c[..., 0] - c[..., 4]).sum(axis=1)  # [O] float64

    xparts = []
    for nb in range(NB):
        xs = x[nb * NSH:(nb + 1) * NSH, :]                 # [NSH, I]
        # [nt, n_in, i_blk, i_in] -> [nt, i_in, i_blk, n_in]
        xp = xs.reshape(NT, 128, IB, 128).transpose(0, 3, 2, 1)
        xparts.append(np.ascontiguousarray(xp.reshape(NT, 128, I), dtype=np.float32))

    wparts, w8parts = [], []
    for ob in range(OB):
        wsl = w_mono[:, ob * OSH:(ob + 1) * OSH, :]        # [4, OSH, I]
        # fp16 tiles: [k, i_in, o]
        wt = np.empty((NK16, 128, OSH), dtype=np.float16)
        for k, (p, ib) in enumerate(K16):
            wt[k] = wsl[p, :, ib * 128:(ib + 1) * 128].T
        wparts.append(wt)
        # fp8 DoubleRow units: [u, i_in, slot*OSH + o]
        w8 = np.empty((NK8, 128, 2 * OSH), dtype=ml_dtypes.float8_e4m3)
        for u, (p, j) in enumerate(K8):
            for s in range(2):
                blk = wsl[p, :, (2 * j + s) * 128:(2 * j + s + 1) * 128].T
                w8[u, :, s * OSH:(s + 1) * OSH] = blk.astype(np.float32)
        w8parts.append(w8)

    return xparts, wparts, w8parts, bias


def _run(x, coeffs, trace=False):
    import os

    from concourse.bass_utils import run_bass_kernel_spmd

    if not trace:
        # A stray BASS_TRACE in the environment would route through the NTFF
        # profile hook, which this image does not ship.
        os.environ["BASS_NEVER_TRACE"] = "1"
    else:
        os.environ.pop("BASS_NEVER_TRACE", None)

    xparts, wparts, w8parts, bias = _prep_inputs(x, coeffs)
    nc = _build_program()
    in_maps = [
        {"xt": xparts[c // OB], "wt": wparts[c % OB], "w8": w8parts[c % OB]}
        for c in range(NB * OB)
    ]
    res = run_bass_kernel_spmd(nc, in_maps, list(range(NB * OB)), trace=trace)

    out = np.empty((N, O), dtype=np.float64)
    for c in range(NB * OB):
        nb, ob = divmod(c, OB)
        out[nb * NSH:(nb + 1) * NSH, ob * OSH:(ob + 1) * OSH] = (
            res.results[c]["out"].reshape(NSH, OSH)
        )
    out += bias[None, :]
    np.clip(out, -10.0, 10.0, out=out)
    return out.astype(np.float32), res


def kernel(x, coeffs):
    return _run(np.asarray(x), np.asarray(coeffs))[0]


# revision 18
# speedup vs baseline: 1.0136x; 1.0047x over previous
"""ChebyKAN layer kernel for 8 Trainium2 NeuronCores.

Reference computation:
    t = tanh(clip(x, -10, 10))                       # [N, I]
    ch = stack([T0(t) .. T4(t)], -1)                  # Chebyshev basis, deg 4
    out = clip(einsum('nid,oid->no', ch, coeffs), -10, 10)

Since t = tanh(.) lies in (-1, 1), every Chebyshev value is in [-1, 1] and the
intermediate clips at +-10 are no-ops.  We rewrite the basis as
{t, v, t*v, v^2} with v = 2t^2-1 on the host:

    out[n,o] = bias[o] + sum_i ( (c1-c3) t + c2 v + 2 c3 (tv) + 2 c4 v^2 )
    bias[o] = sum_i (c0 - c4)[o,i]          (added on host)

which is a [N, 4*I] x [4*I, O] matmul after the elementwise basis prep.

Mixed precision: the tolerance (norm rel err < 2e-2) lets a fraction of the
contraction run as e4m3 DoubleRow matmuls (2 fp8 MACs/cell/cycle).  Measured
quantization noise per plane ranks {v, tv, t, v^2} cheapest-first; 12 of 32
pair-units (plane v entirely + the first half of plane tv) in fp8 lands at
~1.76e-2 total.  The rest stays fp16.  All weights are prescaled by 2^13 so
the fp8 weights sit in e4m3's normal range; the PSUM result is scaled back
by 2^-13 during evacuation (free - the evacuation copy becomes a scaled copy).

Sharding over 8 cores: 4-way over N (rows of x), 2-way over O (output
columns).  Each core holds its W shard resident in SBUF and streams 32
tiles of 128 rows of x, computing tanh + powers on scalar/vector engines
and the matmul on the tensor engine (fp32 PSUM accumulation).
"""

import numpy as np

N, I, O, DEG = 16384, 2048, 2048, 4
NB, OB = 4, 2                      # core grid: 4-way over N, 2-way over O
NSH = N // NB                      # 4096 rows per core
OSH = O // OB                      # 1024 out cols per core
NT = NSH // 128                    # 32 n-tiles per core
IB = I // 128                      # 16 i-blocks

# bf16/fp16 contraction tiles: (plane, i-block) in matmul order
K16 = [(0, ib) for ib in range(IB)] + [(2, ib) for ib in range(14, IB)] + \
      [(3, ib) for ib in range(IB)]                     # 34 tiles
# fp8 DoubleRow pair-units: (plane, pair j) covering i-blocks 2j, 2j+1
K8 = [(1, j) for j in range(8)] + [(2, j) for j in range(7)]  # 15 units
NK16 = len(K16)
NK8 = len(K8)
WS = 2.0 ** 13                     # weight prescale for the fp8 path


def _build_program():
    from concourse.bacc import Bacc
    from concourse.tile import TileContext
    import concourse.mybir as mybir

    f32 = mybir.dt.float32
    f16 = mybir.dt.float16
    f8 = mybir.dt.float8e4
    TANH = mybir.ActivationFunctionType.Tanh
    DR = mybir.MatmulPerfMode.DoubleRow

    nc = Bacc(None, target_bir_lowering=False)
    xt_d = nc.dram_tensor("xt", [NT, 128, I], f16, kind="ExternalInput")
    wt_d = nc.dram_tensor("wt", [NK16, 128, OSH], f16, kind="ExternalInput")
    w8_d = nc.dram_tensor("w8", [NK8, 128, 2 * OSH], f8, kind="ExternalInput")
    out_d = nc.dram_tensor("out", [NT, 128, OSH], f32, kind="ExternalOutput")

    NWARM = 2  # n-tiles processed k-major-interleaved while W streams in
    EVAC = 2.0 ** -13

    with TileContext(nc) as tc:
        with (
            tc.tile_pool(name="wpool", bufs=1) as wpool,
            tc.tile_pool(name="xin", bufs=3) as xpool,
            tc.tile_pool(name="work", bufs=2) as pool,
            tc.tile_pool(name="tpool", bufs=3) as tpool,
            tc.tile_pool(name="psum", bufs=8, space="PSUM") as pp,
        ):
            def load_powers(nt):
                # Basis planes {t, v, t*v, v^2} with v = 2t^2-1: all bounded
                # by 1.  fp16 for the K16 tiles, e4m3 for the K8 units.
                # x ships as f16 (halves the input DMA; tanh error impact is
                # ~1e-4, far below the fp8 noise) and t lives only in f16.
                xt = xpool.tile([128, I], f16, tag="xt")
                nc.sync.dma_start(out=xt[:], in_=xt_d[nt])
                t1 = tpool.tile([128, I], f16, tag="t1")
                nc.scalar.activation(t1[:], xt[:], TANH)
                uv = xpool.tile([128, I], f32, tag="uv")
                nc.vector.tensor_mul(uv[:], t1[:], t1[:])
                # v = 2u - 1, in place (fp32)
                nc.vector.tensor_scalar(
                    uv[:], uv[:], 2.0, -1.0,
                    mybir.AluOpType.mult, mybir.AluOpType.add,
                )
                # plane v entirely in fp8 (pair-sliced 3D for DoubleRow)
                a8v = tpool.tile([128, IB, 128], f8, tag="a8v")
                nc.vector.tensor_copy(a8v[:, :, :], uv[:])
                # plane tv: first 14 i-blocks fp8, rest fp16
                a8tv = tpool.tile([128, 14, 128], f8, tag="a8tv")
                nc.vector.tensor_mul(a8tv[:, :, :], t1[:, 0:1792], uv[:, 0:1792])
                t3 = tpool.tile([128, 256], f16, tag="t3")
                nc.vector.tensor_mul(t3[:], t1[:, 1792:2048], uv[:, 1792:2048])
                # plane v^2 fp16
                t4 = tpool.tile([128, I], f16, tag="t4")
                nc.vector.tensor_mul(t4[:], uv[:], uv[:])
                return {"t1": t1, "t3": t3, "t4": t4, "a8v": a8v, "a8tv": a8tv}

            def lhs16(tp, k):
                p, ib = K16[k]
                if p == 0:
                    return tp["t1"][:, ib * 128:(ib + 1) * 128]
                if p == 2:
                    return tp["t3"][:, (ib - 14) * 128:(ib - 13) * 128]
                return tp["t4"][:, ib * 128:(ib + 1) * 128]

            def lhs8(tp, u):
                p, j = K8[u]
                src = tp["a8v"] if p == 1 else tp["a8tv"]
                return src[:, 2 * j:2 * j + 2, :]

            def mm16(ps, tp, k, oc, start, stop):
                nc.tensor.matmul(
                    ps[:], lhs16(tp, k),
                    wtiles[k][:, oc * 512:(oc + 1) * 512],
                    start=start, stop=stop,
                )

            def mm8(ps, tp, u, oc, start, stop):
                nc.tensor.matmul(
                    ps[:], lhs8(tp, u),
                    w8tiles[u][:, :, oc * 512:(oc + 1) * 512],
                    start=start, stop=stop, perf_mode=DR,
                )

            def store_out_oc(nt, osb, oc):
                nc.sync.dma_start(
                    out=out_d[nt, :, oc * 512:(oc + 1) * 512],
                    in_=osb[:, oc * 512:(oc + 1) * 512],
                )

            # HAM pre-warm sized to ABUT the real stream: the burst must
            # still be running when t1 lands (~15.5us) or the free-running
            # MID window re-throttles the clock gate during the idle gap.
            junk = pool.tile([128, 512], f16, tag="junk")
            nc.vector.memset(junk[:], 0.0)
            ps_j = pp.tile([128, 512], f32, tag="ps")
            NJUNK = 22  # burst ends right at t1-readiness (~13.2us)
            for i in range(NJUNK):
                nc.tensor.matmul(
                    ps_j[:], junk[:, 0:128], junk[:],
                    start=(i == 0), stop=(i == NJUNK - 1),
                )

            # x tiles for the warmup n-tiles first so their DMAs aren't
            # queued behind the ~13 MB of W.
            tps = []
            for wnt in range(NWARM):
                tp_w = load_powers(wnt)
                tps.append(tp_w)

            wtiles = []
            for k in range(NK16):
                w = wpool.tile([128, OSH], f16, tag=f"w{k}")
                nc.sync.dma_start(out=w[:], in_=wt_d[k])
                wtiles.append(w)
            w8tiles = []
            for u in range(NK8):
                w = wpool.tile([128, 2, OSH], f8, tag=f"w8_{u}")
                nc.sync.dma_start(out=w[:, :, :], in_=w8_d[u])
                w8tiles.append(w)

            # Warmup phase: k-major across NWARM*2 psum groups, so the PE has
            # work for every W k-tile as it lands instead of idling until the
            # whole W shard is resident.
            groups = [(nt, oc) for nt in range(NWARM) for oc in range(OSH // 512)]
            pss = {}
            for g in groups:
                ps_tile = pp.tile([128, 512], f32, tag="ps")
                pss[g] = ps_tile
            for k in range(NK16):
                for (nt, oc) in groups:
                    mm16(pss[(nt, oc)], tps[nt], k, oc,
                         start=(k == 0), stop=False)
            for u in range(NK8):
                for (nt, oc) in groups:
                    mm8(pss[(nt, oc)], tps[nt], u, oc,
                        start=False, stop=(u == NK8 - 1))
            # Prep the first steady n-tile BEFORE the warmup evacuations hit
            # the in-order scalar queue: its tanh then runs during the late
            # warmup matmuls instead of queueing behind evacs that wait for
            # them, closing the warmup->steady PE bubble.  The scheduler
            # orders queues by priority, not emission order, so shift this
            # block to just before the warmup matmuls (196 instructions).
            with tc.high_priority(offset=200):
                tps.append(load_powers(NWARM))
            for nt in range(NWARM):
                osb = pool.tile([128, OSH], f32, tag="osb")
                for oc in range(OSH // 512):
                    nc.scalar.mul(osb[:, oc * 512:(oc + 1) * 512],
                                  pss[(nt, oc)][:], EVAC)
                    store_out_oc(nt, osb, oc)

            # Steady state: W fully resident, k-outer so each stationary
            # t-slice feeds both oc matmuls back to back.
            NOC = OSH // 512
            for nt in range(NWARM, NT):
                tp = tps[nt] if nt < len(tps) else load_powers(nt)
                osb = pool.tile([128, OSH], f32, tag="osb")
                pss2 = []
                for _ in range(NOC):
                    ps = pp.tile([128, 512], f32, tag="ps")
                    pss2.append(ps)
                if nt < NT - 1:
                    for k in range(NK16):
                        for oc in range(NOC):
                            mm16(pss2[oc], tp, k, oc, start=(k == 0), stop=False)
                    for u in range(NK8):
                        for oc in range(NOC):
                            mm8(pss2[oc], tp, u, oc, start=False,
                                stop=(u == NK8 - 1))
                    for oc in range(NOC):
                        nc.scalar.mul(osb[:, oc * 512:(oc + 1) * 512],
                                      pss2[oc][:], EVAC)
                        store_out_oc(nt, osb, oc)
                else:
                    # Last tile goes group-sequential: oc0's evacuation and
                    # store hide under oc1's matmuls, shortening the tail.
                    for oc in range(NOC):
                        for k in range(NK16):
                            mm16(pss2[oc], tp, k, oc, start=(k == 0), stop=False)
                        for u in range(NK8):
                            mm8(pss2[oc], tp, u, oc, start=False,
                                stop=(u == NK8 - 1))
                        nc.scalar.mul(osb[:, oc * 512:(oc + 1) * 512],
                                      pss2[oc][:], EVAC)
                        store_out_oc(nt, osb, oc)

    nc.finalize()
    return nc


def _prep_inputs(x, coeffs):
    """Host-side shard prep: transposed/tiled x per N-block, packed fp16 +
    e4m3 weights per O-block, and the T0/bias term."""
    import ml_dtypes

    # Basis on device: {t, v, t*v, v^2} with v = 2t^2 - 1 (= T2).  Then
    # T1 = t, T2 = v, T3 = 2(tv) - t, T4 = 2v^2 - 1, so
    # out = (c0 - c4) + (c1 - c3) t + c2 v + 2 c3 (tv) + 2 c4 v^2.
    c = coeffs.astype(np.float64)
    w_mono = np.stack(
        [
            c[..., 1] - c[..., 3],    # t
            c[..., 2],                # v
            2.0 * c[..., 3],          # t*v
            2.0 * c[..., 4],          # v^2
        ]
    ) * WS  # [4, O, I], prescaled
    bias = (c[..., 0] - c[..., 4]).sum(axis=1)  # [O] float64

    xparts = []
    for nb in range(NB):
        xs = x[nb * NSH:(nb + 1) * NSH, :]                 # [NSH, I]
        # [nt, n_in, i_blk, i_in] -> [nt, i_in, i_blk, n_in]
        xp = xs.reshape(NT, 128, IB, 128).transpose(0, 3, 2, 1)
        xparts.append(np.ascontiguousarray(xp.reshape(NT, 128, I), dtype=np.float16))

    wparts, w8parts = [], []
    for ob in range(OB):
        wsl = w_mono[:, ob * OSH:(ob + 1) * OSH, :]        # [4, OSH, I]
        # fp16 tiles: [k, i_in, o]
        wt = np.empty((NK16, 128, OSH), dtype=np.float16)
        for k, (p, ib) in enumerate(K16):
            wt[k] = wsl[p, :, ib * 128:(ib + 1) * 128].T
        wparts.append(wt)
        # fp8 DoubleRow units: [u, i_in, slot*OSH + o]
        w8 = np.empty((NK8, 128, 2 * OSH), dtype=ml_dtypes.float8_e4m3)
        for u, (p, j) in enumerate(K8):
            for s in range(2):
                blk = wsl[p, :, (2 * j + s) * 128:(2 * j + s + 1) * 128].T
                w8[u, :, s * OSH:(s + 1) * OSH] = blk.astype(np.float32)
        w8parts.append(w8)

    return xparts, wparts, w8parts, bias


def _run(x, coeffs, trace=False):
    import os

    from concourse.bass_utils import run_bass_kernel_spmd

    if not trace:
        # A stray BASS_TRACE in the environment would route through the NTFF
        # profile hook, which this image does not ship.
        os.environ["BASS_NEVER_TRACE"] = "1"
    else:
        os.environ.pop("BASS_NEVER_TRACE", None)

    xparts, wparts, w8parts, bias = _prep_inputs(x, coeffs)
    nc = _build_program()
    in_maps = [
        {"xt": xparts[c // OB], "wt": wparts[c % OB], "w8": w8parts[c % OB]}
        for c in range(NB * OB)
    ]
    res = run_bass_kernel_spmd(nc, in_maps, list(range(NB * OB)), trace=trace)

    out = np.empty((N, O), dtype=np.float64)
    for c in range(NB * OB):
        nb, ob = divmod(c, OB)
        out[nb * NSH:(nb + 1) * NSH, ob * OSH:(ob + 1) * OSH] = (
            res.results[c]["out"].reshape(NSH, OSH)
        )
    out += bias[None, :]
    np.clip(out, -10.0, 10.0, out=out)
    return out.astype(np.float32), res


def kernel(x, coeffs):
    return _run(np.asarray(x), np.asarray(coeffs))[0]
